# revision 16
# baseline (speedup 1.0000x reference)
"""AUGRU cell (attention-gated GRU update) on 8 Trainium2 NeuronCores.

Data-parallel: the batch dim (16384) of x / att_score / hidden is sharded
across 8 cores (2048 rows each); the six 512x512 weight matrices are
replicated.

Per-core dataflow (per 128-row batch tile, 16 tiles):
  zu = x @ W_u + h @ U_u          (PSUM accum, 8 matmuls, f32r fast path)
  zr = x @ W_r + h @ U_r
  xh = x @ W_h ; hu = h @ U_h
  u = att * sigmoid(zu); r = sigmoid(zr)
  hhat = tanh(xh + r * hu)
  out = h + u * (hhat - h)

Matmuls run in float32r (full-rate fp32 PE mode). Every matmul operand is
produced by a DVE tensor_copy (DMA staging -> f32r resident tiles) and
every PSUM bank's last reader is a DVE op, so each self-loading f32r
Matmult carries at most one sync wait (a walrus codegen limit).
"""

import sys

if "/opt/trn_rl_repo" not in sys.path:
    sys.path.insert(0, "/opt/trn_rl_repo")

import numpy as np

NCORES = 8
P = 128
MM_DTYPE = "f32r"  # "f32r" (tf32-class, rel ~1.5e-4) or "bf16" (rel ~2e-3, faster LDW)
WAIT_LIMIT = 7  # walrus setupSyncWait rejects instructions with more waits

_PROGRAM_CACHE = {}


def _split_multi_waits(nc):
    """walrus codegen accepts at most ONE sync wait per instruction (the
    TPB EVENTS struct has a single wait slot and setupSyncWait refuses to
    spill).  Tile's add_semaphores can emit several waits on one
    instruction; hoist all but the last into same-engine no-ops inserted
    immediately before it.  The engine executes the no-ops (each blocking
    on one semaphore) then the instruction — identical semantics."""
    import concourse.mybir as mybir

    for fn in nc.m.functions:
        for blk in fn.blocks:
            insts = blk.instructions
            i = 0
            while i < len(insts):
                inst = insts[i]
                si = inst.sync_info
                if si is not None and len(si.on_wait) > 1:
                    waits = list(si.on_wait)
                    inst.sync_info = mybir.SyncInfo(
                        on_wait=waits[-1:], on_update=list(si.on_update)
                    )
                    for j, w in enumerate(waits[:-1]):
                        nop = mybir.InstNoOp(
                            name=nc.get_next_instruction_name(),
                            sync_info=mybir.SyncInfo(on_wait=[w], on_update=[]),
                            bass_nofuse=True,
                            engine=inst.engine,
                        )
                        nc.register_instruction(nop)
                        insts.insert(i + j, nop)
                    i += len(waits) - 1
                i += 1


def _build_program(D, H, Bc, with_bias, mm_dtype=None):
    import concourse.bass as bass
    import concourse.mybir as mybir
    import concourse.tile as tile
    from concourse.alu_op_type import AluOpType

    f32 = mybir.dt.float32
    f32r = (mybir.dt.bfloat16 if (mm_dtype or MM_DTYPE) == "bf16"
            else mybir.dt.float32r)
    Sig = mybir.ActivationFunctionType.Sigmoid
    Tanh = mybir.ActivationFunctionType.Tanh

    KD = D // P  # K chunks for x-side matmuls
    KH = H // P  # K chunks for h-side matmuls
    TILES = Bc // P
    BCH = 256  # batch-axis chunk for staging the big transposed loads
    NB = Bc // BCH

    nc = bass.Bass()
    xT_p = nc.declare_dram_parameter("xT", [D, Bc], f32r, isOutput=False)
    hT_p = nc.declare_dram_parameter("hT", [H, Bc], f32r, isOutput=False)
    hN_p = nc.declare_dram_parameter("hN", [Bc, H], f32, isOutput=False)
    att_p = nc.declare_dram_parameter("att", [P, TILES], f32, isOutput=False)
    wnames = ("wu", "wr", "wh", "uu", "ur", "uh")
    w_p = {n: nc.declare_dram_parameter(n, [D if n[0] == "w" else H, H], f32r,
                                        isOutput=False) for n in wnames}
    if with_bias:
        b_p = {n: nc.declare_dram_parameter(n, [P, H], f32, isOutput=False)
               for n in ("bub", "brb", "bhb")}
    out_p = nc.declare_dram_parameter("out", [Bc, H], f32, isOutput=True)

    with tile.TileContext(nc) as tc:
        with (
            tc.tile_pool(name="w", bufs=1) as wpool,
            tc.tile_pool(name="dat", bufs=3) as dpool,
            tc.tile_pool(name="ep", bufs=2) as epool,
            tc.tile_pool(name="ps", bufs=2, space="PSUM") as ppool,
        ):
            # Resident operands, DMA'd straight into f32r tiles (the BIR
            # verifier accepts a DMACopy with f32r output as the "rounded"
            # producer; rounding happens at PE ingestion). Chunked DMAs give
            # fine-grained deps via subtile tracking.
            w_sb = {n: wpool.tile([P, KD if n[0] == "w" else KH, H], f32r,
                                  tag=n, name=f"w_{n}") for n in wnames}
            wviews = {n: w_p[n][:].rearrange("(ko ki) n -> ki ko n", ki=P)
                      for n in wnames}
            xT_sb = wpool.tile([P, KD, Bc], f32r, tag="xT")
            hT_sb = wpool.tile([P, KH, Bc], f32r, tag="hT")
            xview = xT_p[:].rearrange("(ko ki) b -> ki ko b", ki=P)
            hview = hT_p[:].rearrange("(ko ki) b -> ki ko b", ki=P)

            def stage_chunk(c):
                bs = slice(c * BCH, (c + 1) * BCH)
                nc.sync.dma_start(xT_sb[:, :, bs], xview[:, :, bs])
                nc.sync.dma_start(hT_sb[:, :, bs], hview[:, :, bs])

            def stage_weight(n, ko):
                nc.sync.dma_start(w_sb[n][:, ko], wviews[n][:, ko])

            # Order the preamble so the first batch tile can start almost
            # immediately: batch-chunk 0 + ki=0 weights first, then the
            # rest interleaved (the PE consumes weights ki-by-ki).
            stage_chunk(0)
            for ko in range(KD):
                for n in ("wh", "wu", "wr"):
                    stage_weight(n, ko)
            stage_chunk(1)
            for ko in range(KH):
                for n in ("uh", "uu", "ur"):
                    stage_weight(n, ko)
            for c in range(2, NB):
                stage_chunk(c)
            w_sb = {(n, ko): w_sb[n][:, ko] for n in wnames
                    for ko in range(KD if n[0] == "w" else KH)}

            att_sb = wpool.tile([P, TILES], f32, tag="att")
            nc.sync.dma_start(att_sb, att_p[:])
            if with_bias:
                b_sb = {}
                for n in ("bub", "brb", "bhb"):
                    t = wpool.tile([P, H], f32, tag=n)
                    nc.sync.dma_start(t, b_p[n][:])
                    b_sb[n] = t

            for t in range(TILES):
                bsl = slice(t * P, (t + 1) * P)
                h_t = dpool.tile([P, H], f32, tag="h")
                nc.sync.dma_start(h_t, hN_p[bsl, :])

                p_zu = ppool.tile([P, H], f32, tag="zu")
                p_zr = ppool.tile([P, H], f32, tag="zr")
                p_xh = ppool.tile([P, H], f32, tag="xh")
                p_hh = ppool.tile([P, H], f32, tag="hh")

                # p_xh group first: the first matmul of a tile may need a
                # fresh xT chunk (DVE tick), and p_xh's PSUM slot is also
                # DVE-released, so its waits merge into one.
                for ki in range(KD):
                    lx = xT_sb[:, ki, bsl]
                    st = ki == 0
                    nc.tensor.matmul(p_xh, lx, w_sb["wh", ki],
                                     start=st, stop=ki == KD - 1)
                    nc.tensor.matmul(p_zu, lx, w_sb["wu", ki], start=st, stop=False)
                    nc.tensor.matmul(p_zr, lx, w_sb["wr", ki], start=st, stop=False)
                for ki in range(KH):
                    lh = hT_sb[:, ki, bsl]
                    last = ki == KH - 1
                    nc.tensor.matmul(p_hh, lh, w_sb["uh", ki],
                                     start=ki == 0, stop=last)
                    nc.tensor.matmul(p_zu, lh, w_sb["uu", ki],
                                     start=False, stop=last)
                    nc.tensor.matmul(p_zr, lh, w_sb["ur", ki],
                                     start=False, stop=last)

                # Epilogue. Each PSUM bank has exactly one releasing engine
                # (zu/zr: ACT sigmoid; xh/hh: DVE), keeping every matmul's
                # wait count at <=1.
                u = epool.tile([P, H], f32, tag="u")
                r = epool.tile([P, H], f32, tag="r")
                g = epool.tile([P, H], f32, tag="g")
                o = epool.tile([P, H], f32, tag="o")

                if with_bias:
                    zu_s = epool.tile([P, H], f32, tag="zu_s")
                    zr_s = epool.tile([P, H], f32, tag="zr_s")
                    nc.vector.tensor_add(zu_s, p_zu, b_sb["bub"])
                    nc.vector.tensor_add(zr_s, p_zr, b_sb["brb"])
                    nc.scalar.activation(u, zu_s, Sig)
                    nc.scalar.activation(r, zr_s, Sig)
                else:
                    nc.scalar.activation(u, p_zu, Sig)
                    nc.scalar.activation(r, p_zr, Sig)
                nc.vector.tensor_mul(g, r, p_hh)       # r * (h @ U_h)
                nc.vector.tensor_add(g, g, p_xh)       # + x @ W_h
                if with_bias:
                    nc.vector.tensor_add(g, g, b_sb["bhb"])
                nc.scalar.activation(g, g, Tanh)       # hhat
                nc.vector.tensor_sub(g, g, h_t)        # hhat - h
                # g = (g * att) * u  == att*sigmoid(zu) * (hhat - h)
                nc.vector.scalar_tensor_tensor(
                    g, g, att_sb[:, t:t + 1], u, AluOpType.mult, AluOpType.mult
                )
                nc.vector.tensor_add(o, g, h_t)        # h + u*(hhat - h)
                nc.sync.dma_start(out_p[bsl, :], o)

    _split_multi_waits(nc)
    return nc


def check_waits(nc):
    """Matmults and Drains may carry at most 1 sync wait on walrus; other
    instruction classes tolerate more (walrus splits them itself)."""
    bad = []
    for fn in nc.m.functions:
        for blk in fn.blocks:
            for inst in blk.instructions:
                si = inst.sync_info
                nw = len(si.on_wait) if si else 0
                kind = type(inst).__name__
                if nw > 1:
                    bad.append((inst.name, kind, nw))
    return bad


def _get_program(D, H, Bc, with_bias):
    key = (D, H, Bc, with_bias, MM_DTYPE)
    if key not in _PROGRAM_CACHE:
        nc = _build_program(D, H, Bc, with_bias)
        bad = check_waits(nc)
        if bad:
            raise RuntimeError(f"instructions over the sync-wait limit: {bad}")
        _PROGRAM_CACHE[key] = nc
    return _PROGRAM_CACHE[key]


def _np32(a):
    return np.ascontiguousarray(np.asarray(a, dtype=np.float32))


def _mm_cast(a):
    if MM_DTYPE == "bf16":
        import ml_dtypes

        return np.ascontiguousarray(a.astype(ml_dtypes.bfloat16))
    return a


def _prepare(x, att_score, hidden, W_u, U_u, b_u, W_r, U_r, b_r, W_h, U_h, b_h):
    x = _np32(x)
    att_score = _np32(att_score)
    hidden = _np32(hidden)
    B, D = x.shape
    H = hidden.shape[1]
    assert B % (NCORES * P) == 0 and D % P == 0 and H % P == 0
    Bc = B // NCORES

    weights = {
        "wu": _np32(W_u), "wr": _np32(W_r), "wh": _np32(W_h),
        "uu": _np32(U_u), "ur": _np32(U_r), "uh": _np32(U_h),
    }
    biases = [_np32(b_u), _np32(b_r), _np32(b_h)]
    with_bias = any(np.any(b) for b in biases)
    cast_weights = {k: _mm_cast(v) for k, v in weights.items()}

    in_maps = []
    for c in range(NCORES):
        sl = slice(c * Bc, (c + 1) * Bc)
        xs, hs, at = x[sl], hidden[sl], att_score[sl]
        m = {
            "xT": _mm_cast(np.ascontiguousarray(xs.T)),
            "hT": _mm_cast(np.ascontiguousarray(hs.T)),
            "hN": np.ascontiguousarray(hs),
            "att": np.ascontiguousarray(at.reshape(Bc // P, P).T),
        }
        m.update(cast_weights)
        if with_bias:
            m["bub"] = np.ascontiguousarray(np.broadcast_to(biases[0], (P, H)))
            m["brb"] = np.ascontiguousarray(np.broadcast_to(biases[1], (P, H)))
            m["bhb"] = np.ascontiguousarray(np.broadcast_to(biases[2], (P, H)))
        in_maps.append(m)

    nc = _get_program(D, H, Bc, with_bias)
    return nc, in_maps


def _run(inputs, trace=False, **trace_kwargs):
    from concourse.bass_utils import run_bass_kernel_spmd

    nc, in_maps = _prepare(**inputs)
    res = run_bass_kernel_spmd(nc, in_maps, list(range(NCORES)), trace=trace,
                               **trace_kwargs)
    out = np.concatenate([res.results[i]["out"] for i in range(NCORES)], axis=0)
    return out, res


def kernel(**inputs):
    out, _ = _run(inputs, trace=False)
    return out


# revision 18
# speedup vs baseline: 1.0816x; 1.0816x over previous
"""AUGRU cell (attention-gated GRU update) on 8 Trainium2 NeuronCores.

Data-parallel: the batch dim (16384) of x / att_score / hidden is sharded
across 8 cores (2048 rows each); the six 512x512 weight matrices are
replicated.

Per-core dataflow (per 128-row batch tile, 16 tiles):
  zu = x @ W_u + h @ U_u          (PSUM accum, 8 matmuls, f32r fast path)
  zr = x @ W_r + h @ U_r
  xh = x @ W_h ; hu = h @ U_h
  u = att * sigmoid(zu); r = sigmoid(zr)
  hhat = tanh(xh + r * hu)
  out = h + u * (hhat - h)

Matmuls run in float32r (full-rate fp32 PE mode). Every matmul operand is
produced by a DVE tensor_copy (DMA staging -> f32r resident tiles) and
every PSUM bank's last reader is a DVE op, so each self-loading f32r
Matmult carries at most one sync wait (a walrus codegen limit).
"""

import os
import sys

if "/opt/trn_rl_repo" not in sys.path:
    sys.path.insert(0, "/opt/trn_rl_repo")

import numpy as np

NCORES = 8
P = 128
MM_DTYPE = os.environ.get("MM_DTYPE", "f32r")  # "f32r" (tf32-class, rel ~1.5e-4) or "bf16" (rel ~2e-3, faster LDW)
WAIT_LIMIT = 7  # walrus setupSyncWait rejects instructions with more waits

_PROGRAM_CACHE = {}


def _split_multi_waits(nc):
    """walrus codegen accepts at most ONE sync wait per instruction (the
    TPB EVENTS struct has a single wait slot and setupSyncWait refuses to
    spill).  Tile's add_semaphores can emit several waits on one
    instruction; hoist all but the last into same-engine no-ops inserted
    immediately before it.  The engine executes the no-ops (each blocking
    on one semaphore) then the instruction — identical semantics."""
    import concourse.mybir as mybir

    for fn in nc.m.functions:
        for blk in fn.blocks:
            insts = blk.instructions
            i = 0
            while i < len(insts):
                inst = insts[i]
                si = inst.sync_info
                if si is not None and len(si.on_wait) > 1:
                    waits = list(si.on_wait)
                    inst.sync_info = mybir.SyncInfo(
                        on_wait=waits[-1:], on_update=list(si.on_update)
                    )
                    for j, w in enumerate(waits[:-1]):
                        nop = mybir.InstNoOp(
                            name=nc.get_next_instruction_name(),
                            sync_info=mybir.SyncInfo(on_wait=[w], on_update=[]),
                            bass_nofuse=True,
                            engine=inst.engine,
                        )
                        nc.register_instruction(nop)
                        insts.insert(i + j, nop)
                    i += len(waits) - 1
                i += 1


def _build_program(D, H, Bc, with_bias, mm_dtype=None):
    import concourse.bass as bass
    import concourse.mybir as mybir
    import concourse.tile as tile
    from concourse.alu_op_type import AluOpType

    f32 = mybir.dt.float32
    bf16_mode = (mm_dtype or MM_DTYPE) == "bf16"
    f32r = mybir.dt.bfloat16 if bf16_mode else mybir.dt.float32r
    stg_dt = mybir.dt.bfloat16 if bf16_mode else f32
    Sig = mybir.ActivationFunctionType.Sigmoid
    Tanh = mybir.ActivationFunctionType.Tanh

    KD = D // P  # K chunks for x-side matmuls
    KH = H // P  # K chunks for h-side matmuls
    TILES = Bc // P
    BCH = 256  # batch-axis chunk for staging the big transposed loads
    NB = Bc // BCH

    nc = bass.Bass()
    xT_p = nc.declare_dram_parameter("xT", [D, Bc], stg_dt, isOutput=False)
    hT_p = nc.declare_dram_parameter("hT", [H, Bc], stg_dt, isOutput=False)
    hN_p = nc.declare_dram_parameter("hN", [Bc, H], f32, isOutput=False)
    att_p = nc.declare_dram_parameter("att", [P, TILES], f32, isOutput=False)
    wnames = ("wu", "wr", "wh", "uu", "ur", "uh")
    w_p = {n: nc.declare_dram_parameter(n, [D if n[0] == "w" else H, H], stg_dt,
                                        isOutput=False) for n in wnames}
    if with_bias:
        b_p = {n: nc.declare_dram_parameter(n, [P, H], f32, isOutput=False)
               for n in ("bub", "brb", "bhb")}
    out_p = nc.declare_dram_parameter("out", [Bc, H], f32, isOutput=True)

    with tile.TileContext(nc) as tc:
        with (
            tc.tile_pool(name="w", bufs=1) as wpool,
            tc.tile_pool(name="stg", bufs=4) as spool,
            tc.tile_pool(name="dat", bufs=3) as dpool,
            tc.tile_pool(name="ep", bufs=2) as epool,
            tc.tile_pool(name="ps", bufs=2, space="PSUM") as ppool,
        ):
            # Resident matmul operands, DMA'd to staging then DVE-copied
            # into f32r/bf16 tiles: a single producing engine for all PE
            # operands keeps matmul wait counts at <=1 (fewer PE-queue
            # no-ops from _split_multi_waits, measurably faster than
            # DMA-direct). Chunked copies give fine-grained deps.
            w_sb = {n: wpool.tile([P, KD if n[0] == "w" else KH, H], f32r,
                                  tag=n, name=f"w_{n}") for n in wnames}
            wviews = {n: w_p[n][:].rearrange("(ko ki) n -> ki ko n", ki=P)
                      for n in wnames}
            xT_sb = wpool.tile([P, KD, Bc], f32r, tag="xT")
            hT_sb = wpool.tile([P, KH, Bc], f32r, tag="hT")
            xview = xT_p[:].rearrange("(ko ki) b -> ki ko b", ki=P)
            hview = hT_p[:].rearrange("(ko ki) b -> ki ko b", ki=P)

            def stage_chunk(c):
                bs = slice(c * BCH, (c + 1) * BCH)
                stg = spool.tile([P, KD, BCH], stg_dt, tag="xs")
                nc.sync.dma_start(stg, xview[:, :, bs])
                nc.vector.tensor_copy(xT_sb[:, :, bs], stg)
                stg = spool.tile([P, KH, BCH], stg_dt, tag="hs")
                nc.sync.dma_start(stg, hview[:, :, bs])
                nc.vector.tensor_copy(hT_sb[:, :, bs], stg)

            def stage_weight(n, ko):
                stg = spool.tile([P, H], stg_dt, tag="ws")
                nc.sync.dma_start(stg, wviews[n][:, ko])
                nc.vector.tensor_copy(w_sb[n][:, ko], stg)

            # Order the preamble so the first batch tile can start almost
            # immediately: batch-chunk 0 + ki=0 weights first, then the
            # rest interleaved (the PE consumes weights ki-by-ki).
            stage_chunk(0)
            for ko in range(KD):
                for n in ("wh", "wu", "wr"):
                    stage_weight(n, ko)
            stage_chunk(1)
            for ko in range(KH):
                for n in ("uh", "uu", "ur"):
                    stage_weight(n, ko)
            for c in range(2, NB):
                stage_chunk(c)
            w_sb = {(n, ko): w_sb[n][:, ko] for n in wnames
                    for ko in range(KD if n[0] == "w" else KH)}

            att_sb = wpool.tile([P, TILES], f32, tag="att")
            nc.sync.dma_start(att_sb, att_p[:])
            if with_bias:
                b_sb = {}
                for n in ("bub", "brb", "bhb"):
                    t = wpool.tile([P, H], f32, tag=n)
                    nc.sync.dma_start(t, b_p[n][:])
                    b_sb[n] = t

            for t in range(TILES):
                bsl = slice(t * P, (t + 1) * P)
                h_t = dpool.tile([P, H], f32, tag="h")
                nc.sync.dma_start(h_t, hN_p[bsl, :])

                p_zu = ppool.tile([P, H], f32, tag="zu")
                p_zr = ppool.tile([P, H], f32, tag="zr")
                p_xh = ppool.tile([P, H], f32, tag="xh")
                p_hh = ppool.tile([P, H], f32, tag="hh")

                # p_xh group first: the first matmul of a tile may need a
                # fresh xT chunk (DVE tick), and p_xh's PSUM slot is also
                # DVE-released, so its waits merge into one.
                for ki in range(KD):
                    lx = xT_sb[:, ki, bsl]
                    st = ki == 0
                    nc.tensor.matmul(p_xh, lx, w_sb["wh", ki],
                                     start=st, stop=ki == KD - 1)
                    nc.tensor.matmul(p_zu, lx, w_sb["wu", ki], start=st, stop=False)
                    nc.tensor.matmul(p_zr, lx, w_sb["wr", ki], start=st, stop=False)
                for ki in range(KH):
                    lh = hT_sb[:, ki, bsl]
                    last = ki == KH - 1
                    nc.tensor.matmul(p_hh, lh, w_sb["uh", ki],
                                     start=ki == 0, stop=last)
                    nc.tensor.matmul(p_zu, lh, w_sb["uu", ki],
                                     start=False, stop=last)
                    nc.tensor.matmul(p_zr, lh, w_sb["ur", ki],
                                     start=False, stop=last)

                # Epilogue. Each PSUM bank has exactly one releasing engine
                # (zu/zr: ACT sigmoid; xh/hh: DVE), keeping every matmul's
                # wait count at <=1.
                u = epool.tile([P, H], f32, tag="u")
                r = epool.tile([P, H], f32, tag="r")
                g = epool.tile([P, H], f32, tag="g")
                o = epool.tile([P, H], f32, tag="o")

                if with_bias:
                    zu_s = epool.tile([P, H], f32, tag="zu_s")
                    zr_s = epool.tile([P, H], f32, tag="zr_s")
                    nc.vector.tensor_add(zu_s, p_zu, b_sb["bub"])
                    nc.vector.tensor_add(zr_s, p_zr, b_sb["brb"])
                    nc.scalar.activation(u, zu_s, Sig)
                    nc.scalar.activation(r, zr_s, Sig)
                else:
                    nc.scalar.activation(u, p_zu, Sig)
                    nc.scalar.activation(r, p_zr, Sig)
                nc.vector.tensor_mul(g, r, p_hh)       # r * (h @ U_h)
                nc.vector.tensor_add(g, g, p_xh)       # + x @ W_h
                if with_bias:
                    nc.vector.tensor_add(g, g, b_sb["bhb"])
                nc.scalar.activation(g, g, Tanh)       # hhat
                nc.vector.tensor_sub(g, g, h_t)        # hhat - h
                # g = (g * att) * u  == att*sigmoid(zu) * (hhat - h)
                nc.vector.scalar_tensor_tensor(
                    g, g, att_sb[:, t:t + 1], u, AluOpType.mult, AluOpType.mult
                )
                nc.vector.tensor_add(o, g, h_t)        # h + u*(hhat - h)
                nc.sync.dma_start(out_p[bsl, :], o)

    _split_multi_waits(nc)
    return nc


def check_waits(nc):
    """Matmults and Drains may carry at most 1 sync wait on walrus; other
    instruction classes tolerate more (walrus splits them itself)."""
    bad = []
    for fn in nc.m.functions:
        for blk in fn.blocks:
            for inst in blk.instructions:
                si = inst.sync_info
                nw = len(si.on_wait) if si else 0
                kind = type(inst).__name__
                if nw > 1:
                    bad.append((inst.name, kind, nw))
    return bad


def _get_program(D, H, Bc, with_bias):
    key = (D, H, Bc, with_bias, MM_DTYPE)
    if key not in _PROGRAM_CACHE:
        nc = _build_program(D, H, Bc, with_bias)
        bad = check_waits(nc)
        if bad:
            raise RuntimeError(f"instructions over the sync-wait limit: {bad}")
        _PROGRAM_CACHE[key] = nc
    return _PROGRAM_CACHE[key]


def _np32(a):
    return np.ascontiguousarray(np.asarray(a, dtype=np.float32))


def _mm_cast(a):
    if MM_DTYPE == "bf16":
        import ml_dtypes

        return np.ascontiguousarray(a.astype(ml_dtypes.bfloat16))
    return a


def _prepare(x, att_score, hidden, W_u, U_u, b_u, W_r, U_r, b_r, W_h, U_h, b_h):
    x = _np32(x)
    att_score = _np32(att_score)
    hidden = _np32(hidden)
    B, D = x.shape
    H = hidden.shape[1]
    assert B % (NCORES * P) == 0 and D % P == 0 and H % P == 0
    Bc = B // NCORES

    weights = {
        "wu": _np32(W_u), "wr": _np32(W_r), "wh": _np32(W_h),
        "uu": _np32(U_u), "ur": _np32(U_r), "uh": _np32(U_h),
    }
    biases = [_np32(b_u), _np32(b_r), _np32(b_h)]
    with_bias = any(np.any(b) for b in biases)
    cast_weights = {k: _mm_cast(v) for k, v in weights.items()}

    in_maps = []
    for c in range(NCORES):
        sl = slice(c * Bc, (c + 1) * Bc)
        xs, hs, at = x[sl], hidden[sl], att_score[sl]
        m = {
            "xT": _mm_cast(np.ascontiguousarray(xs.T)),
            "hT": _mm_cast(np.ascontiguousarray(hs.T)),
            "hN": np.ascontiguousarray(hs),
            "att": np.ascontiguousarray(at.reshape(Bc // P, P).T),
        }
        m.update(cast_weights)
        if with_bias:
            m["bub"] = np.ascontiguousarray(np.broadcast_to(biases[0], (P, H)))
            m["brb"] = np.ascontiguousarray(np.broadcast_to(biases[1], (P, H)))
            m["bhb"] = np.ascontiguousarray(np.broadcast_to(biases[2], (P, H)))
        in_maps.append(m)

    nc = _get_program(D, H, Bc, with_bias)
    return nc, in_maps


def _run(inputs, trace=False, **trace_kwargs):
    from concourse.bass_utils import run_bass_kernel_spmd

    nc, in_maps = _prepare(**inputs)
    res = run_bass_kernel_spmd(nc, in_maps, list(range(NCORES)), trace=trace,
                               **trace_kwargs)
    out = np.concatenate([res.results[i]["out"] for i in range(NCORES)], axis=0)
    return out, res


def kernel(**inputs):
    out, _ = _run(inputs, trace=False)
    return out


# revision 20
# speedup vs baseline: 1.2379x; 1.1445x over previous
"""AUGRU cell (attention-gated GRU update) on 8 Trainium2 NeuronCores.

Data-parallel: the batch dim (16384) of x / att_score / hidden is sharded
across 8 cores (2048 rows each); the six 512x512 weight matrices are
replicated.

Per-core dataflow (per 128-row batch tile, 16 tiles):
  zu = x @ W_u + h @ U_u          (PSUM accum, 8 matmuls, f32r fast path)
  zr = x @ W_r + h @ U_r
  xh = x @ W_h ; hu = h @ U_h
  u = att * sigmoid(zu); r = sigmoid(zr)
  hhat = tanh(xh + r * hu)
  out = h + u * (hhat - h)

Matmuls run in float32r (full-rate fp32 PE mode). Every matmul operand is
produced by a DVE tensor_copy (DMA staging -> f32r resident tiles) and
every PSUM bank's last reader is a DVE op, so each self-loading f32r
Matmult carries at most one sync wait (a walrus codegen limit).
"""

import os
import sys

if "/opt/trn_rl_repo" not in sys.path:
    sys.path.insert(0, "/opt/trn_rl_repo")

import numpy as np

NCORES = 8
P = 128
MM_DTYPE = os.environ.get("MM_DTYPE", "f32r")  # "f32r" (tf32-class, rel ~1.5e-4) or "bf16" (rel ~2e-3, faster LDW)
WAIT_LIMIT = 7  # walrus setupSyncWait rejects instructions with more waits

_PROGRAM_CACHE = {}


def _split_multi_waits(nc):
    """walrus codegen accepts at most ONE sync wait per instruction (the
    TPB EVENTS struct has a single wait slot and setupSyncWait refuses to
    spill).  Tile's add_semaphores can emit several waits on one
    instruction; hoist all but the last into same-engine no-ops inserted
    immediately before it.  The engine executes the no-ops (each blocking
    on one semaphore) then the instruction — identical semantics."""
    import concourse.mybir as mybir

    for fn in nc.m.functions:
        for blk in fn.blocks:
            insts = blk.instructions
            i = 0
            while i < len(insts):
                inst = insts[i]
                si = inst.sync_info
                if si is not None and len(si.on_wait) > 1:
                    waits = list(si.on_wait)
                    inst.sync_info = mybir.SyncInfo(
                        on_wait=waits[-1:], on_update=list(si.on_update)
                    )
                    for j, w in enumerate(waits[:-1]):
                        nop = mybir.InstNoOp(
                            name=nc.get_next_instruction_name(),
                            sync_info=mybir.SyncInfo(on_wait=[w], on_update=[]),
                            bass_nofuse=True,
                            engine=inst.engine,
                        )
                        nc.register_instruction(nop)
                        insts.insert(i + j, nop)
                    i += len(waits) - 1
                i += 1


def _build_program(D, H, Bc, with_bias, mm_dtype=None):
    import concourse.bass as bass
    import concourse.mybir as mybir
    import concourse.tile as tile
    from concourse.alu_op_type import AluOpType

    f32 = mybir.dt.float32
    bf16_mode = (mm_dtype or MM_DTYPE) == "bf16"
    f32r = mybir.dt.bfloat16 if bf16_mode else mybir.dt.float32r
    stg_dt = mybir.dt.bfloat16 if bf16_mode else f32
    Sig = mybir.ActivationFunctionType.Sigmoid
    Tanh = mybir.ActivationFunctionType.Tanh

    KD = D // P  # K chunks for x-side matmuls
    KH = H // P  # K chunks for h-side matmuls
    TILES = Bc // P
    BCH = 256  # batch-axis chunk for staging the big transposed loads
    NB = Bc // BCH

    nc = bass.Bass()
    xT_p = nc.declare_dram_parameter("xT", [D, Bc], stg_dt, isOutput=False)
    hT_p = nc.declare_dram_parameter("hT", [H, Bc], stg_dt, isOutput=False)
    hN_p = nc.declare_dram_parameter("hN", [Bc, H], f32, isOutput=False)
    att_p = nc.declare_dram_parameter("att", [P, TILES], f32, isOutput=False)
    wnames = ("wu", "wr", "wh", "uu", "ur", "uh")
    w_p = {n: nc.declare_dram_parameter(n, [D if n[0] == "w" else H, H], stg_dt,
                                        isOutput=False) for n in wnames}
    if with_bias:
        b_p = {n: nc.declare_dram_parameter(n, [P, H], f32, isOutput=False)
               for n in ("bub", "brb", "bhb")}
    out_p = nc.declare_dram_parameter("out", [Bc, H], f32, isOutput=True)

    with tile.TileContext(nc) as tc:
        with (
            tc.tile_pool(name="w", bufs=1) as wpool,
            tc.tile_pool(name="stg", bufs=4) as spool,
            tc.tile_pool(name="dat", bufs=3) as dpool,
            tc.tile_pool(name="ep", bufs=2) as epool,
            tc.tile_pool(name="ps", bufs=2, space="PSUM") as ppool,
        ):
            # Resident matmul operands, DMA'd to staging then DVE-copied
            # into f32r/bf16 tiles: a single producing engine for all PE
            # operands keeps matmul wait counts at <=1 (fewer PE-queue
            # no-ops from _split_multi_waits, measurably faster than
            # DMA-direct). Chunked copies give fine-grained deps.
            w_sb = {n: wpool.tile([P, KD if n[0] == "w" else KH, H], f32r,
                                  tag=n, name=f"w_{n}") for n in wnames}
            wviews = {n: w_p[n][:].rearrange("(ko ki) n -> ki ko n", ki=P)
                      for n in wnames}
            xT_sb = wpool.tile([P, KD, Bc], f32r, tag="xT")
            hT_sb = wpool.tile([P, KH, Bc], f32r, tag="hT")
            xview = xT_p[:].rearrange("(ko ki) b -> ki ko b", ki=P)
            hview = hT_p[:].rearrange("(ko ki) b -> ki ko b", ki=P)

            def stage_chunk(lo, size):
                bs = slice(lo, lo + size)
                stg = spool.tile([P, KD, BCH], stg_dt, tag="xs")
                nc.sync.dma_start(stg[:, :, :size], xview[:, :, bs])
                nc.vector.tensor_copy(xT_sb[:, :, bs], stg[:, :, :size])
                stg = spool.tile([P, KH, BCH], stg_dt, tag="hs")
                nc.sync.dma_start(stg[:, :, :size], hview[:, :, bs])
                nc.vector.tensor_copy(hT_sb[:, :, bs], stg[:, :, :size])

            def stage_weight(n, ko):
                stg = spool.tile([P, H], stg_dt, tag="ws")
                nc.sync.dma_start(stg, wviews[n][:, ko])
                nc.vector.tensor_copy(w_sb[n][:, ko], stg)

            # PE warm-up: the HAM clock gate needs ~3.4us of sustained
            # PE activity before it lifts the array clock from 1.2 to
            # 2.4 GHz. Junk bf16 weight loads (legal standalone, unlike
            # f32r) keep the PE busy while the first DMAs land, so the
            # real matmuls start warm.
            warm = wpool.tile([P, P], mybir.dt.bfloat16, tag="warm")
            nc.vector.memset(warm, 0.0)
            for _ in range(32):
                nc.tensor.ldweights(warm)

            # Order the preamble by first consumption: a small first
            # batch chunk, then weights in the exact order tile 0 uses
            # them (x-side ki=0..3, then h-side ki=0..3), then the rest.
            stage_chunk(0, P)
            for ko in range(KD):
                for n in ("wh", "wu", "wr"):
                    stage_weight(n, ko)
            for ko in range(KH):
                for n in ("uh", "uu", "ur"):
                    stage_weight(n, ko)
            stage_chunk(P, P)
            for c in range(1, NB):
                stage_chunk(c * BCH, BCH)
            w_sb = {(n, ko): w_sb[n][:, ko] for n in wnames
                    for ko in range(KD if n[0] == "w" else KH)}

            att_sb = wpool.tile([P, TILES], f32, tag="att")
            nc.sync.dma_start(att_sb, att_p[:])
            if with_bias:
                b_sb = {}
                for n in ("bub", "brb", "bhb"):
                    t = wpool.tile([P, H], f32, tag=n)
                    nc.sync.dma_start(t, b_p[n][:])
                    b_sb[n] = t

            for t in range(TILES):
                bsl = slice(t * P, (t + 1) * P)
                h_t = dpool.tile([P, H], f32, tag="h")
                nc.sync.dma_start(h_t, hN_p[bsl, :])

                p_zu = ppool.tile([P, H], f32, tag="zu")
                p_zr = ppool.tile([P, H], f32, tag="zr")
                p_xh = ppool.tile([P, H], f32, tag="xh")
                p_hh = ppool.tile([P, H], f32, tag="hh")

                # p_xh group first: the first matmul of a tile may need a
                # fresh xT chunk (DVE tick), and p_xh's PSUM slot is also
                # DVE-released, so its waits merge into one.
                for ki in range(KD):
                    lx = xT_sb[:, ki, bsl]
                    st = ki == 0
                    nc.tensor.matmul(p_xh, lx, w_sb["wh", ki],
                                     start=st, stop=ki == KD - 1)
                    nc.tensor.matmul(p_zu, lx, w_sb["wu", ki], start=st, stop=False)
                    nc.tensor.matmul(p_zr, lx, w_sb["wr", ki], start=st, stop=False)
                for ki in range(KH):
                    lh = hT_sb[:, ki, bsl]
                    last = ki == KH - 1
                    nc.tensor.matmul(p_hh, lh, w_sb["uh", ki],
                                     start=ki == 0, stop=last)
                    nc.tensor.matmul(p_zu, lh, w_sb["uu", ki],
                                     start=False, stop=last)
                    nc.tensor.matmul(p_zr, lh, w_sb["ur", ki],
                                     start=False, stop=last)

                # Epilogue. Each PSUM bank has exactly one releasing engine
                # (zu/zr: ACT sigmoid; xh/hh: DVE), keeping every matmul's
                # wait count at <=1.
                u = epool.tile([P, H], f32, tag="u")
                r = epool.tile([P, H], f32, tag="r")
                g = epool.tile([P, H], f32, tag="g")
                o = epool.tile([P, H], f32, tag="o")

                if with_bias:
                    zu_s = epool.tile([P, H], f32, tag="zu_s")
                    zr_s = epool.tile([P, H], f32, tag="zr_s")
                    nc.vector.tensor_add(zu_s, p_zu, b_sb["bub"])
                    nc.vector.tensor_add(zr_s, p_zr, b_sb["brb"])
                    nc.scalar.activation(u, zu_s, Sig)
                    nc.scalar.activation(r, zr_s, Sig)
                else:
                    nc.scalar.activation(u, p_zu, Sig)
                    nc.scalar.activation(r, p_zr, Sig)
                nc.vector.tensor_mul(g, r, p_hh)       # r * (h @ U_h)
                nc.vector.tensor_add(g, g, p_xh)       # + x @ W_h
                if with_bias:
                    nc.vector.tensor_add(g, g, b_sb["bhb"])
                nc.scalar.activation(g, g, Tanh)       # hhat
                nc.vector.tensor_sub(g, g, h_t)        # hhat - h
                # g = (g * att) * u  == att*sigmoid(zu) * (hhat - h)
                nc.vector.scalar_tensor_tensor(
                    g, g, att_sb[:, t:t + 1], u, AluOpType.mult, AluOpType.mult
                )
                nc.vector.tensor_add(o, g, h_t)        # h + u*(hhat - h)
                nc.sync.dma_start(out_p[bsl, :], o)

    _split_multi_waits(nc)
    return nc


def check_waits(nc):
    """Matmults and Drains may carry at most 1 sync wait on walrus; other
    instruction classes tolerate more (walrus splits them itself)."""
    bad = []
    for fn in nc.m.functions:
        for blk in fn.blocks:
            for inst in blk.instructions:
                si = inst.sync_info
                nw = len(si.on_wait) if si else 0
                kind = type(inst).__name__
                if nw > 1:
                    bad.append((inst.name, kind, nw))
    return bad


def _get_program(D, H, Bc, with_bias):
    key = (D, H, Bc, with_bias, MM_DTYPE)
    if key not in _PROGRAM_CACHE:
        nc = _build_program(D, H, Bc, with_bias)
        bad = check_waits(nc)
        if bad:
            raise RuntimeError(f"instructions over the sync-wait limit: {bad}")
        _PROGRAM_CACHE[key] = nc
    return _PROGRAM_CACHE[key]


def _np32(a):
    return np.ascontiguousarray(np.asarray(a, dtype=np.float32))


def _mm_cast(a):
    if MM_DTYPE == "bf16":
        import ml_dtypes

        return np.ascontiguousarray(a.astype(ml_dtypes.bfloat16))
    return a


def _prepare(x, att_score, hidden, W_u, U_u, b_u, W_r, U_r, b_r, W_h, U_h, b_h):
    x = _np32(x)
    att_score = _np32(att_score)
    hidden = _np32(hidden)
    B, D = x.shape
    H = hidden.shape[1]
    assert B % (NCORES * P) == 0 and D % P == 0 and H % P == 0
    Bc = B // NCORES

    weights = {
        "wu": _np32(W_u), "wr": _np32(W_r), "wh": _np32(W_h),
        "uu": _np32(U_u), "ur": _np32(U_r), "uh": _np32(U_h),
    }
    biases = [_np32(b_u), _np32(b_r), _np32(b_h)]
    with_bias = any(np.any(b) for b in biases)
    cast_weights = {k: _mm_cast(v) for k, v in weights.items()}

    in_maps = []
    for c in range(NCORES):
        sl = slice(c * Bc, (c + 1) * Bc)
        xs, hs, at = x[sl], hidden[sl], att_score[sl]
        m = {
            "xT": _mm_cast(np.ascontiguousarray(xs.T)),
            "hT": _mm_cast(np.ascontiguousarray(hs.T)),
            "hN": np.ascontiguousarray(hs),
            "att": np.ascontiguousarray(at.reshape(Bc // P, P).T),
        }
        m.update(cast_weights)
        if with_bias:
            m["bub"] = np.ascontiguousarray(np.broadcast_to(biases[0], (P, H)))
            m["brb"] = np.ascontiguousarray(np.broadcast_to(biases[1], (P, H)))
            m["bhb"] = np.ascontiguousarray(np.broadcast_to(biases[2], (P, H)))
        in_maps.append(m)

    nc = _get_program(D, H, Bc, with_bias)
    return nc, in_maps


def _run(inputs, trace=False, **trace_kwargs):
    from concourse.bass_utils import run_bass_kernel_spmd

    nc, in_maps = _prepare(**inputs)
    res = run_bass_kernel_spmd(nc, in_maps, list(range(NCORES)), trace=trace,
                               **trace_kwargs)
    out = np.concatenate([res.results[i]["out"] for i in range(NCORES)], axis=0)
    return out, res


def kernel(**inputs):
    out, _ = _run(inputs, trace=False)
    return out
